# revision 1
# baseline (speedup 1.0000x reference)
"""Paged-attention decode (vLLM single_query_cached_kv_attention +
reshape_and_cache) for Trainium2, 8 NeuronCores.

Strategy
--------
Sequences are sharded across the 8 cores (4 per core), sorted by context
length so each "slot" (per-core sequence index) has a similar length on
every core; one SPMD program is built with a per-slot group count
G = ceil((L-1)/128) taken as the max over the 8 cores of that slot.

Per (slot, head-half) on each core, a K tile [128=(block_lane nb,
d_outer do), 8 heads x G*128] is loaded with one DMA per (nb, group) --
the DMA-queue cost is a fixed ~630ns per instruction, so DMAs batch 8
heads each (the 3-dim AP limit forbids batching more dims). V tiles are
per-head [(g,pos), (nb,d)]. DMAs round-robin across all three DGE rings
(sync/scalar HWDGE + gpsimd SWDGE).

Per (slot, head):
  * Scores: 8 accumulating float32r TensorE matmuls (one per d_inner di)
    whose stationary operand is a block-diagonal matrix holding SCALE*q
    (8 diagonal copies of q[do,di], one per block lane) - contracting
    (nb,do) partitions exactly, with no transposes or casts of K.
  * Softmax: the tail mask (pos > L-2; slot L-1 holds the stale value the
    reference overwrites) is applied with one affine_select; ACT computes
    exp with fused per-lane row sums.
  * attn^T via one small PE transpose; V accumulated with 8 fp32r matmuls
    against the naturally-laid-out V tile.
  * The new token's k/v (reshape_and_cache) is folded in exactly via a
    batched side path: e_new = exp(SCALE*q.k_new) joins the softmax sum
    and e_new*v_new joins the output accumulation.
Outputs are normalized by the reciprocal of the exp-sum and gathered.
"""
import sys

for _p in ("/opt/trn_rl_repo", "/root/.axon_site/_ro/trn_rl_repo"):
    if _p not in sys.path:
        sys.path.insert(0, _p)

import numpy as np
import concourse.bass as bass
import concourse.mybir as mybir
import concourse.tile as tile
from concourse.bass_utils import run_bass_kernel_spmd

F32 = mybir.dt.float32
F32R = mybir.dt.float32r
AF = mybir.ActivationFunctionType
ALU = mybir.AluOpType

SCALE = 0.08838834764831845  # 1/sqrt(128)
B, H, D, BS, NB, X, MAX_BLOCKS = 32, 16, 128, 16, 2048, 8, 64
N_CORES = 8
SLOTS = B // N_CORES  # 4
HH = 8  # heads per K tile


def split_multi_waits(nc):
    """This walrus build rejects instructions with more than one sync wait;
    move extra waits onto preceding same-engine NoOps (equivalent: an
    engine's queue executes sequentially, so a wait on the NoOp still
    gates the following instruction)."""
    for f in nc.m.functions:
        for blk in f.blocks:
            new = []
            for ins in blk.instructions:
                si = ins.sync_info
                if si is not None and len(si.on_wait) > 1:
                    waits = list(si.on_wait)
                    for w in waits[:-1]:
                        nop = mybir.InstNoOp(
                            name=f"waitsplit-{nc.next_id()}",
                            engine=ins.engine, ins=[], outs=[])
                        nop.sync_info = mybir.SyncInfo(on_wait=[w], on_update=[])
                        new.append(nop)
                    si.on_wait = waits[-1:]
                new.append(ins)
            blk.instructions = new


def build_program(G_slots, lens_max, n_heads=H, dev_sim=False):
    """Single SPMD program. G_slots[s] = #groups of (8 blocks x 16 pos);
    lens_max[s] = the slot's max context length (for the mask base)."""
    n_slots = len(G_slots)
    nblk_tot = 8 * sum(G_slots)
    offs = np.cumsum([0] + [8 * g for g in G_slots])
    NSH = n_slots * n_heads
    hper = min(HH, n_heads)
    hh_cnt = (n_heads + hper - 1) // hper

    nc = bass.Bass()
    kc = nc.declare_dram_parameter("kc", [nblk_tot, n_heads, 2048], F32, isOutput=False)
    vc = nc.declare_dram_parameter("vc", [nblk_tot, n_heads, 16, 128], F32, isOutput=False)
    bdq = nc.declare_dram_parameter("bdq", [128, NSH * 64], F32R, isOutput=False)
    qh = nc.declare_dram_parameter("qh", [NSH, 128], F32, isOutput=False)
    kn = nc.declare_dram_parameter("kn", [NSH, 128], F32, isOutput=False)
    vn = nc.declare_dram_parameter("vn", [1, NSH * 128], F32R, isOutput=False)
    msk = nc.declare_dram_parameter("msk", [8, n_slots * 128], F32, isOutput=False)
    ident = nc.declare_dram_parameter("ident", [64, 64], F32, isOutput=False)
    ones = nc.declare_dram_parameter("ones", [64, 1], F32, isOutput=False)
    out = nc.declare_dram_parameter("out", [1, NSH * 128], F32, isOutput=True)

    rings = (nc.sync, nc.scalar, nc.gpsimd)

    with tile.TileContext(nc) as tc:
        with (
            tc.tile_pool(name="const", bufs=1) as cpool,
            tc.tile_pool(name="kx", bufs=2) as kpool,
            tc.tile_pool(name="vx", bufs=4) as vpool,
            tc.tile_pool(name="sm", bufs=4) as spool,
            tc.tile_pool(name="ps_s", bufs=3, space="PSUM") as ps_s_pool,
            tc.tile_pool(name="ps_t", bufs=2, space="PSUM") as ps_t_pool,
            tc.tile_pool(name="ps_o", bufs=3, space="PSUM") as ps_o_pool,
        ):
            # ---- constants + batched new-token side path ----
            t_bdq = cpool.tile([128, NSH * 64], F32R, tag="bdq")
            nc.sync.dma_start(t_bdq[:], bdq[:])
            t_id = cpool.tile([64, 64], F32, tag="ident")
            nc.scalar.dma_start(t_id[:], ident[:])
            t_ones = cpool.tile([64, 1], F32, tag="ones")
            nc.scalar.dma_start(t_ones[:], ones[:])
            t_vn = cpool.tile([1, NSH * 128], F32R, tag="vn")
            nc.sync.dma_start(t_vn[:], vn[:])
            t_msk = cpool.tile([8, n_slots * 128], F32, tag="msk")
            nc.scalar.dma_start(t_msk[:], msk[:])
            t_qh = cpool.tile([NSH, 128], F32, tag="qh")
            nc.scalar.dma_start(t_qh[:], qh[:])
            t_kn = cpool.tile([NSH, 128], F32, tag="kn")
            nc.scalar.dma_start(t_kn[:], kn[:])

            t_prod = cpool.tile([NSH, 128], F32, tag="prod")
            t_snew = cpool.tile([NSH, 1], F32, tag="snew")
            nc.vector.tensor_mul(t_prod[:], t_qh[:], t_kn[:])
            nc.vector.reduce_sum(t_snew[:], t_prod[:], axis=mybir.AxisListType.X)
            t_enew = cpool.tile([NSH, 1], F32, tag="enew")
            nc.scalar.activation(t_enew[:], t_snew[:], AF.Exp, scale=SCALE)
            ps_en = ps_s_pool.tile([8, 128], F32, tag="scores")
            nc.tensor.transpose(ps_en[0:1, 0:NSH], t_enew[:], t_id[0:NSH, 0:NSH])
            t_enew_r = cpool.tile([1, NSH], F32, tag="enewr")
            nc.vector.tensor_copy(t_enew_r[:], ps_en[0:1, 0:NSH])
            t_enew_rr = cpool.tile([1, NSH], F32R, tag="enewrr")
            nc.vector.tensor_copy(t_enew_rr[:], ps_en[0:1, 0:NSH])

            t_out = cpool.tile([1, NSH * 128], F32, tag="outrow")

            ring_i = 0
            # ---- per (slot, head-half) ----
            for s in range(n_slots):
                G = G_slots[s]
                blk0 = int(offs[s])
                Lm = int(lens_max[s])
                for hh in range(hh_cnt):
                    h0 = hh * hper
                    # K tile [128=(nb,do), hper*(G*128)], free = (h, g, pos*di)
                    t_k = kpool.tile([128, hper * G * 128], F32R, tag="ktile")
                    if dev_sim:
                        nc.gpsimd.memset(t_k[:], 0.0)
                    k_view = t_k[:].rearrange(
                        "k (h g pd) -> k h g pd", h=hper, g=G)
                    for nb in range(8):
                        for g in range(G):
                            kin = kc[blk0 + 8 * g + nb, h0:h0 + hper, :].bitcast(
                                F32R).rearrange("h (do pd) -> do h pd", pd=128)
                            kout = k_view[nb * 16:(nb + 1) * 16, :, g, :]
                            rings[ring_i % 3].dma_start(kout, kin)
                            ring_i += 1
                    k_r = t_k[:].rearrange(
                        "k (h g p di) -> k h g p di", h=hper, p=16, di=8)

                    for hl in range(hper):
                        h = h0 + hl
                        sh = s * n_heads + h
                        # V tile [(g,pos)=G*16, (nb,d)=1024]
                        t_v = vpool.tile([G * 16, 8 * 128], F32R, tag="vtile")
                        if dev_sim:
                            nc.gpsimd.memset(t_v[:], 0.0)
                        for g in range(G):
                            vin = vc[blk0 + 8 * g: blk0 + 8 * g + 8, h, :, :].bitcast(
                                F32R).rearrange("nb p d -> p nb d")
                            vout_ = t_v[g * 16:(g + 1) * 16, :].rearrange(
                                "p (nb d) -> p nb d", d=128)
                            rings[ring_i % 3].dma_start(vout_, vin)
                            ring_i += 1

                        # scores [8, G*16] = 8 accumulating di-matmuls
                        ps_sc = ps_s_pool.tile([8, 128], F32, tag="scores")
                        for di in range(8):
                            nc.tensor.matmul(
                                ps_sc[:, 0:G * 16],
                                t_bdq[:, sh * 64 + di * 8: sh * 64 + di * 8 + 8],
                                k_r[:, hl, :, :, di],
                                start=(di == 0), stop=(di == 7))

                        # masked scores: mask is per (core, slot) data
                        # (-1e9 where p > L-2), shared by all heads of a slot
                        t_m2 = spool.tile([8, 128], F32, tag="msc2")
                        nc.vector.tensor_add(
                            t_m2[:, 0:G * 16], ps_sc[:, 0:G * 16],
                            t_msk[:, s * 128: s * 128 + G * 16])
                        t_ex = spool.tile([8, 128], F32, tag="exps")
                        t_sum = spool.tile([8, 1], F32, tag="sums")
                        nc.scalar.activation(
                            t_ex[:, 0:G * 16], t_m2[:, 0:G * 16], AF.Exp,
                            accum_out=t_sum[:])

                        # attn^T [G*16, 8]
                        ps_at = ps_t_pool.tile([128, 8], F32, tag="attnT")
                        nc.tensor.transpose(ps_at[0:G * 16, :], t_ex[:, 0:G * 16],
                                            t_id[0:8, 0:8])
                        t_at = spool.tile([128, 8], F32R, tag="attnTs")
                        nc.vector.tensor_copy(t_at[0:G * 16, :], ps_at[0:G * 16, :])

                        # out_unnorm [1,128] and total [1,1] share one PSUM tile
                        ps_o = ps_o_pool.tile([1, 256], F32, tag="vout")
                        v_r = t_v[:].rearrange("gp (nb d) -> gp nb d", nb=8)
                        for nb in range(8):
                            nc.tensor.matmul(
                                ps_o[:, 0:128],
                                t_at[0:G * 16, nb:nb + 1],
                                v_r[:, nb, :],
                                start=(nb == 0), stop=False, skip_group_check=True)
                        nc.tensor.matmul(
                            ps_o[:, 0:128],
                            t_enew_rr[:, sh:sh + 1],
                            t_vn[:, sh * 128:(sh + 1) * 128],
                            start=False, stop=True, skip_group_check=True)
                        nc.tensor.matmul(
                            ps_o[:, 128:129], t_ones[0:8, :], t_sum[:],
                            start=True, stop=False, skip_group_check=True)
                        nc.tensor.matmul(
                            ps_o[:, 128:129], t_enew_r[:, sh:sh + 1], t_ones[0:1, :],
                            start=False, stop=True, skip_group_check=True)

                        t_rec = spool.tile([1, 1], F32, tag="rec")
                        nc.vector.reciprocal(t_rec[:], ps_o[:, 128:129])
                        nc.vector.tensor_scalar_mul(
                            t_out[:, sh * 128:(sh + 1) * 128], ps_o[:, 0:128],
                            t_rec[:])

            nc.sync.dma_start(out[:], t_out[:])

    return nc


def _host_inputs(G_slots, seq_ids_by_core, query, key, value, key_cache,
                 value_cache, block_tables, context_lens):
    """Per-core input maps. seq_ids_by_core[c][s] = sequence index."""
    n_slots = len(G_slots)
    NSH = n_slots * H
    key_cache = np.asarray(key_cache)
    value_cache = np.asarray(value_cache)
    block_tables = np.asarray(block_tables)
    query = np.asarray(query)
    key = np.asarray(key)
    value = np.asarray(value)

    ident = np.eye(64, dtype=np.float32)
    ones_arr = np.ones((64, 1), np.float32)
    context_lens = np.asarray(context_lens)

    g_idx = np.arange(8)
    pos_idx = np.arange(16)
    nb_idx = np.arange(8)
    p_grid = (g_idx[None, :, None] * 8 + nb_idx[:, None, None]) * 16 \
        + pos_idx[None, None, :]  # (nb, g, pos)

    in_maps = []
    for c in range(N_CORES):
        ids = seq_ids_by_core[c]
        blk_rows = np.concatenate(
            [block_tables[ids[s], 0:8 * G_slots[s]] for s in range(n_slots)])
        kc = key_cache[blk_rows]          # [N, H, 16, 16, 8]
        vc = value_cache[blk_rows]        # [N, H, 16, 128]
        kc = np.ascontiguousarray(kc.reshape(kc.shape[0], H, 2048))
        vc = np.ascontiguousarray(vc)

        q_rows = query[ids]               # [n_slots, H, 128]
        kn_rows = key[ids]
        vn_rows = value[ids]

        qv = q_rows.reshape(n_slots, H, 16, 8)
        bdq = np.zeros((8, 16, n_slots, H, 8, 8), np.float32)
        for nb in range(8):
            bdq[nb, :, :, :, :, nb] = SCALE * qv.transpose(2, 0, 1, 3)
        bdq = np.ascontiguousarray(bdq.reshape(128, NSH * 64))

        msk = np.empty((8, n_slots, 8, 16), np.float32)
        for s in range(n_slots):
            L = int(context_lens[ids[s]])
            msk[:, s] = np.where(p_grid <= L - 2, 0.0, -1e9)
        msk = np.ascontiguousarray(msk.reshape(8, n_slots * 128))

        in_maps.append(dict(
            kc=kc, vc=vc, bdq=bdq, msk=msk,
            qh=np.ascontiguousarray(q_rows.reshape(NSH, 128)),
            kn=np.ascontiguousarray(kn_rows.reshape(NSH, 128)),
            vn=np.ascontiguousarray(vn_rows.reshape(1, NSH * 128)),
            ident=ident, ones=ones_arr,
        ))
    return in_maps


def _plan(context_lens):
    """Assign sequences to (core, slot) sorted by length; per-slot G."""
    lens = np.asarray(context_lens)
    order = np.argsort(-lens, kind="stable")  # longest first
    seq_ids_by_core = [[0] * SLOTS for _ in range(N_CORES)]
    G_slots = []
    lens_max = []
    for s in range(SLOTS):
        chunk = order[s * N_CORES:(s + 1) * N_CORES]
        for c in range(N_CORES):
            seq_ids_by_core[c][s] = int(chunk[c])
        Lmax = int(lens[chunk].max())
        lens_max.append(Lmax)
        G_slots.append(max(1, -(-(Lmax - 1) // 128)))  # ceil((L-1)/128)
    return tuple(G_slots), lens_max, seq_ids_by_core


def kernel(query, key, value, key_cache, value_cache, block_tables,
           context_lens, slot_mapping, _run=None):
    G_slots, lens_max, seq_ids_by_core = _plan(context_lens)
    nc = build_program(G_slots, lens_max)
    split_multi_waits(nc)
    in_maps = _host_inputs(G_slots, seq_ids_by_core, query, key, value,
                           key_cache, value_cache, block_tables, context_lens)
    runner = _run or (lambda nc_, maps: run_bass_kernel_spmd(
        nc_, maps, core_ids=list(range(N_CORES))).results)
    results = runner(nc, in_maps)

    out = np.empty((B, H * D), np.float32)
    for c in range(N_CORES):
        row = results[c]["out"].reshape(SLOTS * H * D)
        for s in range(SLOTS):
            i = seq_ids_by_core[c][s]
            out[i] = row[s * H * D:(s + 1) * H * D]
    return out



# revision 4
# speedup vs baseline: 3.8084x; 3.8084x over previous
"""Paged-attention decode (vLLM single_query_cached_kv_attention +
reshape_and_cache) for Trainium2, 8 NeuronCores.

Strategy
--------
Sequences are sharded across the 8 cores (4 per core), sorted by context
length so each "slot" (per-core sequence index) has a similar length on
every core; one SPMD program is built with a per-slot chunk count
G = ceil(L/128) taken as the max over the 8 cores of that slot.

The host gathers each slot's KV blocks, applies reshape_and_cache (the
new token's k/v written at position L-1), zeroes V rows at invalid
positions, appends a "ones" column to V (position-validity indicator so
the softmax denominator falls out of the same matmuls that compute the
output), casts to bf16 and lays the tiles out in DRAM exactly as SBUF
wants them:
  K^T per slot: [128 = d, (head, chunk, pos128)]
  V   per slot: [128 = pos%128, (head, chunk, 129 = d+ones)]
so each slot loads with 4 large (~1-4 MB) DMAs at near-peak HBM
bandwidth instead of hundreds of 64 KB descriptor-bound transfers.

Per (slot, head): G score matmuls (stationary = K^T chunk [128d x
128pos], moving = scaled q column) put positions on PSUM partitions;
one ACT Exp produces bf16 exp-scores [128, G] (no mask needed: invalid
positions have zeroed V and ones-column); G accumulating AV matmuls
(stationary = exp column, moving = V chunk [128pos, 129]) yield the
unnormalized output and the exp-sum in one PSUM row [1, 129]; DVE
reciprocal + scalar-mul normalize. No transposes anywhere.
"""
import sys

for _p in ("/opt/trn_rl_repo", "/root/.axon_site/_ro/trn_rl_repo"):
    if _p not in sys.path:
        sys.path.insert(0, _p)

import numpy as np
import ml_dtypes
import concourse.bass as bass
import concourse.mybir as mybir
import concourse.tile as tile
from concourse.bass_utils import run_bass_kernel_spmd

F32 = mybir.dt.float32
BF16 = mybir.dt.bfloat16
AF = mybir.ActivationFunctionType

SCALE = 0.08838834764831845  # 1/sqrt(128)
B, H, D, BS, NB, X, MAX_BLOCKS = 32, 16, 128, 16, 2048, 8, 64
N_CORES = 8
SLOTS = B // N_CORES  # 4
DP = D + 1  # V free size: 128 dims + ones column


def split_multi_waits(nc):
    """This walrus build rejects instructions with more than one sync wait;
    move extra waits onto preceding same-engine NoOps (equivalent: an
    engine's queue executes sequentially, so a wait on the NoOp still
    gates the following instruction)."""
    for f in nc.m.functions:
        for blk in f.blocks:
            new = []
            for ins in blk.instructions:
                si = ins.sync_info
                if si is not None and len(si.on_wait) > 1:
                    waits = list(si.on_wait)
                    for w in waits[:-1]:
                        nop = mybir.InstNoOp(
                            name=f"waitsplit-{nc.next_id()}",
                            engine=ins.engine, ins=[], outs=[])
                        nop.sync_info = mybir.SyncInfo(on_wait=[w], on_update=[])
                        new.append(nop)
                    si.on_wait = waits[-1:]
                new.append(ins)
            blk.instructions = new


def build_program(G_slots, n_heads=H):
    """Single SPMD program. G_slots[s] = #chunks of 128 positions."""
    n_slots = len(G_slots)
    NSH = n_slots * n_heads
    sumG = sum(G_slots)

    nc = bass.Bass()
    kt = nc.declare_dram_parameter("kt", [128, n_heads * 128 * sumG], BF16,
                                   isOutput=False)
    vt = nc.declare_dram_parameter("vt", [128, n_heads * DP * sumG], BF16,
                                   isOutput=False)
    qt = nc.declare_dram_parameter("qt", [128, NSH], BF16, isOutput=False)
    out = nc.declare_dram_parameter("out", [1, NSH * 128], F32, isOutput=True)

    rings = (nc.sync, nc.scalar, nc.gpsimd)
    ring_i = 0

    with tile.TileContext(nc) as tc:
        with (
            tc.tile_pool(name="const", bufs=1) as cpool,
            tc.tile_pool(name="kx", bufs=2) as kpool,
            tc.tile_pool(name="vx", bufs=2) as vpool,
            tc.tile_pool(name="ex", bufs=4) as epool,
            tc.tile_pool(name="rx", bufs=4) as rpool,
            tc.tile_pool(name="ps_s", bufs=4, space="PSUM") as ps_s_pool,
            tc.tile_pool(name="ps_o", bufs=4, space="PSUM") as ps_o_pool,
        ):
            t_qt = cpool.tile([128, NSH], BF16, tag="qt")
            nc.sync.dma_start(t_qt[:], qt[:])
            t_out = cpool.tile([1, NSH * 128], F32, tag="outrow")

            koff = 0
            voff = 0
            for s in range(n_slots):
                G = G_slots[s]
                kw = n_heads * 128 * G
                vw = n_heads * DP * G
                t_k = kpool.tile([128, kw], BF16, tag="ktile")
                t_v = vpool.tile([128, vw], BF16, tag="vtile")
                # head-major free layout: halves split by head so compute
                # on heads 0-7 starts after the first half lands
                for half, (t, src, off, w) in enumerate(
                        ((t_k, kt, koff, kw), (t_v, vt, voff, vw))):
                    rings[ring_i % 3].dma_start(
                        t[:, 0:w // 2], src[:, off:off + w // 2])
                    ring_i += 1
                    rings[ring_i % 3].dma_start(
                        t[:, w // 2:w], src[:, off + w // 2:off + w])
                    ring_i += 1
                koff += kw
                voff += vw

                # software-pipeline: scores for head h, AV for head h-2
                pend = {}
                for idx in range(n_heads + 2):
                    if idx < n_heads:
                        h = idx
                        sh = s * n_heads + h
                        ps = ps_s_pool.tile([128, 8], F32, tag="sc")
                        for c in range(G):
                            o = (h * G + c) * 128
                            nc.tensor.matmul(
                                ps[:, c:c + 1], t_k[:, o:o + 128],
                                t_qt[:, sh:sh + 1], start=True, stop=True)
                        t_e = epool.tile([128, 8], BF16, tag="e")
                        nc.scalar.activation(t_e[:, 0:G], ps[:, 0:G], AF.Exp)
                        pend[h] = t_e
                    if idx >= 2:
                        h = idx - 2
                        sh = s * n_heads + h
                        t_e = pend.pop(h)
                        po = ps_o_pool.tile([1, DP], F32, tag="o")
                        for c in range(G):
                            o = (h * G + c) * DP
                            nc.tensor.matmul(
                                po[:], t_e[:, c:c + 1], t_v[:, o:o + DP],
                                start=(c == 0), stop=(c == G - 1),
                                skip_group_check=True)
                        t_rec = rpool.tile([1, 1], F32, tag="rec")
                        nc.vector.reciprocal(t_rec[:], po[:, 128:129])
                        nc.vector.tensor_scalar_mul(
                            t_out[:, sh * 128:(sh + 1) * 128],
                            po[:, 0:128], t_rec[:])

            nc.sync.dma_start(out[:], t_out[:])

    return nc


def _host_inputs(G_slots, seq_ids_by_core, query, key, value, key_cache,
                 value_cache, block_tables, context_lens):
    """Per-core input maps. seq_ids_by_core[c][s] = sequence index."""
    n_slots = len(G_slots)
    NSH = n_slots * H
    sumG = sum(G_slots)
    key_cache = np.asarray(key_cache)
    value_cache = np.asarray(value_cache)
    block_tables = np.asarray(block_tables)
    query = np.asarray(query)
    key = np.asarray(key)
    value = np.asarray(value)
    context_lens = np.asarray(context_lens)
    bf = ml_dtypes.bfloat16

    in_maps = []
    for c in range(N_CORES):
        ids = seq_ids_by_core[c]
        kt = np.empty((128, H * 128 * sumG), dtype=bf)
        vt = np.empty((128, H * DP * sumG), dtype=bf)
        koff = 0
        voff = 0
        for s in range(n_slots):
            G = G_slots[s]
            i = int(ids[s])
            L = int(context_lens[i])
            P = G * 128
            blocks = block_tables[i, 0:8 * G]
            # [8G, H, 16do, 16bs, 8x] -> [P, H, 128]
            kb = key_cache[blocks]
            k_seq = np.ascontiguousarray(
                kb.transpose(0, 3, 1, 2, 4)).reshape(P, H, D)
            vb = value_cache[blocks]
            v_seq = np.ascontiguousarray(
                vb.transpose(0, 2, 1, 3)).reshape(P, H, D).copy()
            # reshape_and_cache: the new token lives at position L-1
            k_seq[L - 1] = key[i]
            v_seq[L - 1] = value[i]
            v_seq[L:] = 0.0
            # K^T tile [d, (h, chunk, pos)]
            ktile = k_seq.reshape(G, 128, H, D).transpose(3, 2, 0, 1)
            kt[:, koff:koff + H * 128 * G] = \
                ktile.reshape(D, H * G * 128).astype(bf)
            # V tile [pos%128, (h, chunk, d+ones)]
            vtile = np.empty((128, H, G, DP), np.float32)
            vtile[:, :, :, 0:D] = v_seq.reshape(G, 128, H, D).transpose(
                1, 2, 0, 3)
            ones = (np.arange(P) < L).astype(np.float32).reshape(G, 128)
            vtile[:, :, :, D] = ones.T[:, None, :]
            vt[:, voff:voff + H * DP * G] = \
                vtile.reshape(128, H * G * DP).astype(bf)
            koff += H * 128 * G
            voff += H * DP * G

        q_rows = query[ids]  # [n_slots, H, 128]
        qt = (q_rows.reshape(NSH, D).T * np.float32(SCALE)).astype(bf)
        in_maps.append(dict(kt=kt, vt=vt, qt=np.ascontiguousarray(qt)))
    return in_maps


def _plan(context_lens):
    """Assign sequences to (core, slot) sorted by length; per-slot G.
    Slots ordered shortest-first so the first (unoverlapped) DMA is the
    smallest."""
    lens = np.asarray(context_lens)
    order = np.argsort(-lens, kind="stable")  # longest first
    seq_ids_by_core = [[0] * SLOTS for _ in range(N_CORES)]
    G_slots = []
    for s in range(SLOTS):
        chunk = order[s * N_CORES:(s + 1) * N_CORES]
        for c in range(N_CORES):
            seq_ids_by_core[c][s] = int(chunk[c])
        Lmax = int(lens[chunk].max())
        G_slots.append(max(1, -(-Lmax // 128)))  # ceil(L/128)
    # reorder slots shortest-first
    perm = sorted(range(SLOTS), key=lambda s: G_slots[s])
    G_slots = [G_slots[s] for s in perm]
    seq_ids_by_core = [[seq_ids_by_core[c][s] for s in perm]
                       for c in range(N_CORES)]
    return tuple(G_slots), seq_ids_by_core


def kernel(query, key, value, key_cache, value_cache, block_tables,
           context_lens, slot_mapping, _run=None):
    G_slots, seq_ids_by_core = _plan(context_lens)
    nc = build_program(G_slots)
    split_multi_waits(nc)
    in_maps = _host_inputs(G_slots, seq_ids_by_core, query, key, value,
                           key_cache, value_cache, block_tables, context_lens)
    runner = _run or (lambda nc_, maps: run_bass_kernel_spmd(
        nc_, maps, core_ids=list(range(N_CORES))).results)
    results = runner(nc, in_maps)

    out = np.empty((B, H * D), np.float32)
    for c in range(N_CORES):
        row = results[c]["out"].reshape(SLOTS * H * D)
        for s in range(SLOTS):
            i = seq_ids_by_core[c][s]
            out[i] = row[s * H * D:(s + 1) * H * D]
    return out


# revision 6
# speedup vs baseline: 4.8372x; 1.2701x over previous
"""Paged-attention decode (vLLM single_query_cached_kv_attention +
reshape_and_cache) for Trainium2, 8 NeuronCores.

Strategy
--------
Sequences are sharded across the 8 cores (4 per core), sorted by context
length so each "slot" (per-core sequence index) has a similar length on
every core; one SPMD program is built with a per-slot chunk count
G = ceil(L/128) taken as the max over the 8 cores of that slot.

The host gathers each slot's KV blocks, applies reshape_and_cache (the
new token's k/v written at position L-1), zeroes V rows at invalid
positions, appends a "ones" column to V (position-validity indicator so
the softmax denominator falls out of the same matmuls that compute the
output), casts to bf16 and lays the tiles out in DRAM exactly as SBUF
wants them:
  K^T per slot: [128 = d, (head, chunk, pos128)]
  V   per slot: [128 = pos%128, (head, chunk, 129 = d+ones)]
so each slot loads with 4 large (~1-4 MB) DMAs at near-peak HBM
bandwidth instead of hundreds of 64 KB descriptor-bound transfers.

Per (slot, head): G score matmuls (stationary = K^T chunk [128d x
128pos], moving = scaled q column) put positions on PSUM partitions;
one ACT Exp produces bf16 exp-scores [128, G] (no mask needed: invalid
positions have zeroed V and ones-column); G accumulating AV matmuls
(stationary = exp column, moving = V chunk [128pos, 129]) yield the
unnormalized output and the exp-sum in one PSUM row [1, 129]; DVE
reciprocal + scalar-mul normalize. No transposes anywhere.
"""
import sys

for _p in ("/opt/trn_rl_repo", "/root/.axon_site/_ro/trn_rl_repo"):
    if _p not in sys.path:
        sys.path.insert(0, _p)

import numpy as np
import ml_dtypes
import concourse.bass as bass
import concourse.mybir as mybir
import concourse.tile as tile
from concourse.bass_utils import run_bass_kernel_spmd

F32 = mybir.dt.float32
BF16 = mybir.dt.bfloat16
AF = mybir.ActivationFunctionType

SCALE = 0.08838834764831845  # 1/sqrt(128)
B, H, D, BS, NB, X, MAX_BLOCKS = 32, 16, 128, 16, 2048, 8, 64
N_CORES = 8
SLOTS = B // N_CORES  # 4
DP = D + 1  # V free size: 128 dims + ones column


def split_multi_waits(nc):
    """This walrus build rejects instructions with more than one sync wait;
    move extra waits onto preceding same-engine NoOps (equivalent: an
    engine's queue executes sequentially, so a wait on the NoOp still
    gates the following instruction)."""
    for f in nc.m.functions:
        for blk in f.blocks:
            new = []
            for ins in blk.instructions:
                si = ins.sync_info
                if si is not None and len(si.on_wait) > 1:
                    waits = list(si.on_wait)
                    for w in waits[:-1]:
                        nop = mybir.InstNoOp(
                            name=f"waitsplit-{nc.next_id()}",
                            engine=ins.engine, ins=[], outs=[])
                        nop.sync_info = mybir.SyncInfo(on_wait=[w], on_update=[])
                        new.append(nop)
                    si.on_wait = waits[-1:]
                new.append(ins)
            blk.instructions = new


def build_program(G_slots, n_heads=H):
    """Single SPMD program. G_slots[s] = #chunks of 128 positions."""
    n_slots = len(G_slots)
    NSH = n_slots * n_heads
    sumG = sum(G_slots)

    nc = bass.Bass()
    kt = nc.declare_dram_parameter("kt", [128, n_heads * 128 * sumG], BF16,
                                   isOutput=False)
    vt = nc.declare_dram_parameter("vt", [128, n_heads * DP * sumG], BF16,
                                   isOutput=False)
    qt = nc.declare_dram_parameter("qt", [128, NSH], BF16, isOutput=False)
    out = nc.declare_dram_parameter("out", [1, NSH * 128], F32, isOutput=True)

    rings = (nc.sync, nc.scalar)
    ring_i = 0
    NQ = 4  # DMA chunks per slot tile (4 heads each)

    with tile.TileContext(nc) as tc:
        with (
            tc.tile_pool(name="const", bufs=1) as cpool,
            tc.tile_pool(name="kx", bufs=2) as kpool,
            tc.tile_pool(name="vx", bufs=2) as vpool,
            tc.tile_pool(name="ex", bufs=4) as epool,
            tc.tile_pool(name="rx", bufs=4) as rpool,
            tc.tile_pool(name="ps_s", bufs=4, space="PSUM") as ps_s_pool,
            tc.tile_pool(name="ps_o", bufs=4, space="PSUM") as ps_o_pool,
        ):
            t_qt = cpool.tile([128, NSH], BF16, tag="qt")
            nc.sync.dma_start(t_qt[:], qt[:])
            t_out = cpool.tile([1, NSH * 128], F32, tag="outrow")

            koff = 0
            voff = 0
            for s in range(n_slots):
                G = G_slots[s]
                kw = n_heads * 128 * G
                vw = n_heads * DP * G
                t_k = kpool.tile([128, kw], BF16, tag="ktile")
                t_v = vpool.tile([128, vw], BF16, tag="vtile")
                # head-major free layout, loaded in NQ interleaved K/V
                # chunks (head-group granularity) so compute on the first
                # head group starts after 1/NQ of the slot's bytes land
                for q in range(NQ):
                    for t, src, off, w in ((t_k, kt, koff, kw),
                                           (t_v, vt, voff, vw)):
                        a, b = q * w // NQ, (q + 1) * w // NQ
                        rings[ring_i % len(rings)].dma_start(
                            t[:, a:b], src[:, off + a:off + b])
                        ring_i += 1
                koff += kw
                voff += vw

                # software-pipeline: scores for head h, AV for head h-2
                pend = {}
                for idx in range(n_heads + 2):
                    if idx < n_heads:
                        h = idx
                        sh = s * n_heads + h
                        ps = ps_s_pool.tile([128, 8], F32, tag="sc")
                        for c in range(G):
                            o = (h * G + c) * 128
                            nc.tensor.matmul(
                                ps[:, c:c + 1], t_k[:, o:o + 128],
                                t_qt[:, sh:sh + 1], start=True, stop=True)
                        t_e = epool.tile([128, 8], BF16, tag="e")
                        nc.scalar.activation(t_e[:, 0:G], ps[:, 0:G], AF.Exp)
                        pend[h] = t_e
                    if idx >= 2:
                        h = idx - 2
                        sh = s * n_heads + h
                        t_e = pend.pop(h)
                        po = ps_o_pool.tile([1, DP], F32, tag="o")
                        for c in range(G):
                            o = (h * G + c) * DP
                            nc.tensor.matmul(
                                po[:], t_e[:, c:c + 1], t_v[:, o:o + DP],
                                start=(c == 0), stop=(c == G - 1),
                                skip_group_check=True)
                        t_rec = rpool.tile([1, 1], F32, tag="rec")
                        nc.vector.reciprocal(t_rec[:], po[:, 128:129])
                        nc.vector.tensor_scalar_mul(
                            t_out[:, sh * 128:(sh + 1) * 128],
                            po[:, 0:128], t_rec[:])

            nc.sync.dma_start(out[:], t_out[:])

    return nc


def _host_inputs(G_slots, seq_ids_by_core, query, key, value, key_cache,
                 value_cache, block_tables, context_lens):
    """Per-core input maps. seq_ids_by_core[c][s] = sequence index."""
    n_slots = len(G_slots)
    NSH = n_slots * H
    sumG = sum(G_slots)
    key_cache = np.asarray(key_cache)
    value_cache = np.asarray(value_cache)
    block_tables = np.asarray(block_tables)
    query = np.asarray(query)
    key = np.asarray(key)
    value = np.asarray(value)
    context_lens = np.asarray(context_lens)
    bf = ml_dtypes.bfloat16

    in_maps = []
    for c in range(N_CORES):
        ids = seq_ids_by_core[c]
        kt = np.empty((128, H * 128 * sumG), dtype=bf)
        vt = np.empty((128, H * DP * sumG), dtype=bf)
        koff = 0
        voff = 0
        for s in range(n_slots):
            G = G_slots[s]
            i = int(ids[s])
            L = int(context_lens[i])
            P = G * 128
            blocks = block_tables[i, 0:8 * G]
            # [8G, H, 16do, 16bs, 8x] -> [P, H, 128]
            kb = key_cache[blocks]
            k_seq = np.ascontiguousarray(
                kb.transpose(0, 3, 1, 2, 4)).reshape(P, H, D)
            vb = value_cache[blocks]
            v_seq = np.ascontiguousarray(
                vb.transpose(0, 2, 1, 3)).reshape(P, H, D).copy()
            # reshape_and_cache: the new token lives at position L-1
            k_seq[L - 1] = key[i]
            v_seq[L - 1] = value[i]
            v_seq[L:] = 0.0
            # K^T tile [d, (h, chunk, pos)]
            ktile = k_seq.reshape(G, 128, H, D).transpose(3, 2, 0, 1)
            kt[:, koff:koff + H * 128 * G] = \
                ktile.reshape(D, H * G * 128).astype(bf)
            # V tile [pos%128, (h, chunk, d+ones)]
            vtile = np.empty((128, H, G, DP), np.float32)
            vtile[:, :, :, 0:D] = v_seq.reshape(G, 128, H, D).transpose(
                1, 2, 0, 3)
            ones = (np.arange(P) < L).astype(np.float32).reshape(G, 128)
            vtile[:, :, :, D] = ones.T[:, None, :]
            vt[:, voff:voff + H * DP * G] = \
                vtile.reshape(128, H * G * DP).astype(bf)
            koff += H * 128 * G
            voff += H * DP * G

        q_rows = query[ids]  # [n_slots, H, 128]
        qt = (q_rows.reshape(NSH, D).T * np.float32(SCALE)).astype(bf)
        in_maps.append(dict(kt=kt, vt=vt, qt=np.ascontiguousarray(qt)))
    return in_maps


def _plan(context_lens):
    """Assign sequences to (core, slot) sorted by length; per-slot G.
    Slots ordered shortest-first so the first (unoverlapped) DMA is the
    smallest."""
    lens = np.asarray(context_lens)
    order = np.argsort(-lens, kind="stable")  # longest first
    seq_ids_by_core = [[0] * SLOTS for _ in range(N_CORES)]
    G_slots = []
    for s in range(SLOTS):
        chunk = order[s * N_CORES:(s + 1) * N_CORES]
        for c in range(N_CORES):
            seq_ids_by_core[c][s] = int(chunk[c])
        Lmax = int(lens[chunk].max())
        G_slots.append(max(1, -(-Lmax // 128)))  # ceil(L/128)
    # reorder slots shortest-first
    perm = sorted(range(SLOTS), key=lambda s: G_slots[s])
    G_slots = [G_slots[s] for s in perm]
    seq_ids_by_core = [[seq_ids_by_core[c][s] for s in perm]
                       for c in range(N_CORES)]
    return tuple(G_slots), seq_ids_by_core


def kernel(query, key, value, key_cache, value_cache, block_tables,
           context_lens, slot_mapping, _run=None):
    G_slots, seq_ids_by_core = _plan(context_lens)
    nc = build_program(G_slots)
    split_multi_waits(nc)
    in_maps = _host_inputs(G_slots, seq_ids_by_core, query, key, value,
                           key_cache, value_cache, block_tables, context_lens)
    runner = _run or (lambda nc_, maps: run_bass_kernel_spmd(
        nc_, maps, core_ids=list(range(N_CORES))).results)
    results = runner(nc, in_maps)

    out = np.empty((B, H * D), np.float32)
    for c in range(N_CORES):
        row = results[c]["out"].reshape(SLOTS * H * D)
        for s in range(SLOTS):
            i = seq_ids_by_core[c][s]
            out[i] = row[s * H * D:(s + 1) * H * D]
    return out
